# revision 8
# baseline (speedup 1.0000x reference)
"""TP(heads)xDP(batch) sharded causal GQA attention block for 8 trn2 cores.

Each core c handles batch b=c//4 and head group g=c%4 (8 q heads, 2 kv heads).
v2 pipeline (software-pipelined one chunk ahead, PE-stream interleaved):
  slot s emits: attention(s) iterations interleaved with qkv(s+1) matmul
  chunks and o_proj(s-1) tiles, so the PE never waits on RoPE/exp/norm.
  - RoPE rows are pre-permuted on the host so the half-swap is a single
    full-width (128-partition) stream_shuffle; all rope math in bf16.
  - scores_T[kj, qi] = k^T q  (bf16, 2 heads packed via PE row tiling)
  - exp on ScalarE (no max subtraction; scores ~N(0,1) after 1/8 scale),
    diagonal blocks N-restricted to the causal region.
  - out_aug[65, qi] = [v; ones]^T exp  (bf16; row 64 = softmax denominator)
  - normalize via DVE reciprocal_approx_fast + gpsimd partition_broadcast
  - partial_T[dout, t] = Wo_c^T attn (bf16 out); host sums the 8 partials.
"""
import sys
sys.path.insert(0, "/opt/trn_rl_repo")
from contextlib import ExitStack

import numpy as np
import ml_dtypes

B, L, D = 2, 2048, 2048
NH, NKV, HD = 32, 8, 64
ROPE_BASE = 10000.0
SCALE = HD ** -0.5
TC, TCW = 4, 512      # query/token chunks
NKD = 16              # d contraction tiles
NOT = 6               # output tiles per core (4 q packs, k pack, v pack)
NKJ = 16              # key tiles

BF16 = ml_dtypes.bfloat16

_cached = {}

# swap halves of 16 within each 32-partition quadrant (rope pair swap after
# the host-side row permutation)
SWAP16 = [(i + 16) % 32 for i in range(32)]


def _build_nc(dbg=False):
    import concourse.bacc as bacc
    import concourse.tile as tile
    import concourse.mybir as mybir
    from concourse import library_config

    F32 = mybir.dt.float32
    BF = mybir.dt.bfloat16
    AF = mybir.ActivationFunctionType

    nc = bacc.Bacc("TRN2", debug=False)
    xh_ap = nc.dram_tensor("xh", (TC, 128, NKD * TCW), BF, kind="ExternalInput").ap()
    wq_ap = nc.dram_tensor("wq", (128, NKD * NOT * 128), BF, kind="ExternalInput").ap()
    wo_ap = nc.dram_tensor("wo", (128, 4 * NKD * 128), BF, kind="ExternalInput").ap()
    ccss_ap = nc.dram_tensor("ccss", (128, 2 * L), BF, kind="ExternalInput").ap()
    msk_ap = nc.dram_tensor("msk", (128, 4 * TCW), BF, kind="ExternalInput").ap()
    id2_ap = nc.dram_tensor("id2", (128, 64), BF, kind="ExternalInput").ap()
    out_ap = nc.dram_tensor("outp", (TC, NKD, 128, TCW), BF, kind="ExternalOutput").ap()
    if dbg:
        dbg_q0 = nc.dram_tensor("dbg_q0", (128, 4 * TCW), BF, kind="ExternalOutput").ap()
        dbg_k = nc.dram_tensor("dbg_k", (128, L), BF, kind="ExternalOutput").ap()
        dbg_v = nc.dram_tensor("dbg_v", (128, 2 * NKJ * 65), BF, kind="ExternalOutput").ap()
        dbg_ep = nc.dram_tensor("dbg_ep", (128, 2 * TCW), BF, kind="ExternalOutput").ap()
        dbg_aug = nc.dram_tensor("dbg_aug", (65, TCW), F32, kind="ExternalOutput").ap()
        dbg_pair = nc.dram_tensor("dbg_pair", (128, TCW), BF, kind="ExternalOutput").ap()

    with tile.TileContext(nc) as tcx, ExitStack() as ctx:
        pc = ctx.enter_context(tcx.tile_pool(name="const", bufs=1))
        px = ctx.enter_context(tcx.tile_pool(name="x", bufs=2))
        pw = ctx.enter_context(tcx.tile_pool(name="work", bufs=1))
        psc = ctx.enter_context(tcx.tile_pool(name="psc", bufs=2, space="PSUM"))
        paug = ctx.enter_context(tcx.tile_pool(name="paug", bufs=1, space="PSUM"))
        pmm = ctx.enter_context(tcx.tile_pool(name="pmm", bufs=2, space="PSUM"))

        wq_t = pc.tile([128, NKD * NOT * 128], BF)
        wqw = NKD * NOT * 128 // 8
        for i in range(8):
            nc.sync.dma_start(wq_t[:, i * wqw:(i + 1) * wqw], wq_ap[:, i * wqw:(i + 1) * wqw])
        wo_t = pc.tile([128, 4 * NKD * 128], BF)
        wow = 4 * NKD * 128 // 4
        for i in range(4):
            nc.sync.dma_start(wo_t[:, i * wow:(i + 1) * wow], wo_ap[:, i * wow:(i + 1) * wow])
        ccss_t = pc.tile([128, 2 * L], BF)
        for i in range(4):
            nc.sync.dma_start(ccss_t[:, i * L // 2:(i + 1) * L // 2],
                              ccss_ap[:, i * L // 2:(i + 1) * L // 2])
        msk_t = pc.tile([128, 4 * TCW], BF)
        nc.sync.dma_start(msk_t[:, :], msk_ap[:, :])
        id2_t = pc.tile([128, 64], BF)
        nc.sync.dma_start(id2_t[:, :], id2_ap[:, :])

        kpack = pc.tile([128, L], BF)            # k (2 kv heads stacked), RoPE'd
        vaug = pc.tile([128, 2 * NKJ * 65], BF)  # [v | ones] per (kv, kj)
        nc.vector.memset(vaug[:, 64::65], 1.0)   # ones columns

        with tcx.tile_critical():
            nc.gpsimd.load_library(library_config.attn)

        qropes = [None] * TC   # per-chunk roped q tiles
        _pairs = [None] * 4
        prev_pairs = [None]    # pairs of chunk s-1, consumed by o_proj in slot s

        def emit_x_dma(tci):
            xt = px.tile([128, NKD * TCW], BF, tag="xt")
            xw = NKD * TCW // 8
            with tcx.high_priority():
                for i in range(8):
                    nc.sync.dma_start(xt[:, i * xw:(i + 1) * xw],
                                      xh_ap[tci][:, i * xw:(i + 1) * xw])
            return xt

        def rope(dest_ap, raw, tci, nrep, w):
            """dest = raw*CC + swap16(raw)*SS over [128, w]; bf16 throughout."""
            with tcx.high_priority():
                swp = pw.tile([128, 4 * TCW], BF, tag="swp", bufs=2)
                nc.vector.stream_shuffle(swp[:, 0:w], raw[:, 0:w], SWAP16)
                cs = ccss_t[:, tci * TCW:(tci + 1) * TCW].unsqueeze(1).broadcast_to([128, nrep, TCW])
                sss = ccss_t[:, L + tci * TCW:L + (tci + 1) * TCW].unsqueeze(1).broadcast_to([128, nrep, TCW])
                r3 = raw[:, 0:w].rearrange("p (a b) -> p a b", a=nrep)
                s3 = swp[:, 0:w].rearrange("p (a b) -> p a b", a=nrep)
                nc.vector.tensor_mul(r3, r3, cs)
                nc.vector.tensor_mul(s3, s3, sss)
                nc.vector.tensor_add(dest_ap.rearrange("p (a b) -> p a b", a=nrep), r3, s3)

        def qkv_chunks(tci):
            """Return list of closures emitting the qkv projection for chunk
            tci: per-ot matmul groups (k, v first), then the q rope."""
            xt = emit_x_dma(tci)
            qraw = pw.tile([128, 4 * TCW], BF, tag="qraw", bufs=2)
            chunks = []

            def ot_chunk(ot):
                def emit():
                    ps = pmm.tile([128, TCW], F32, tag="mm")
                    for dt in range(NKD):
                        nc.tensor.matmul(
                            ps[:, :], wq_t[:, (dt * NOT + ot) * 128:(dt * NOT + ot + 1) * 128],
                            xt[:, dt * TCW:(dt + 1) * TCW],
                            start=(dt == 0), stop=(dt == NKD - 1))
                    if ot == 4:
                        kraw = pw.tile([128, TCW], BF, tag="kraw", bufs=2)
                        with tcx.high_priority():
                            nc.vector.tensor_copy(kraw[:, :], ps[:, :])
                        rope(kpack[:, tci * TCW:(tci + 1) * TCW], kraw, tci, 1, TCW)
                    elif ot == 5:
                        vch = pw.tile([128, TCW], BF, tag="vch", bufs=2)
                        with tcx.high_priority():
                            nc.vector.tensor_copy(vch[:, :], ps[:, :])
                        for j in range(2):
                            for jj in range(4):
                                kj = 4 * tci + jj
                                tp = pmm.tile([128, 64], BF, tag="mm")
                                nc.tensor.transpose(
                                    tp[:, :], vch[64 * j:64 * j + 64, jj * 128:(jj + 1) * 128],
                                    id2_t[64 * j:64 * j + 64, :])
                                col = (j * NKJ + kj) * 65
                                with tcx.high_priority():
                                    nc.vector.tensor_copy(vaug[:, col:col + 64], tp[:, :])
                    else:
                        with tcx.high_priority():
                            nc.vector.tensor_copy(qraw[:, ot * TCW:(ot + 1) * TCW], ps[:, :])
                return emit

            for ot in (4, 5, 0, 1, 2, 3):
                chunks.append(ot_chunk(ot))
            chunks.append(lambda: rope(qraw[:, :], qraw, tci, 4, 4 * TCW))
            qropes[tci] = qraw
            return chunks

        def oproj_chunks(otc, pairs_):
            """16 closures, one per dout tile (4 matmuls each)."""
            def dt_chunk(dt):
                def emit():
                    po = pmm.tile([128, TCW], F32, tag="mm")
                    for kt in range(4):
                        nc.tensor.matmul(
                            po[:, :], wo_t[:, (kt * NKD + dt) * 128:(kt * NKD + dt + 1) * 128],
                            pairs_[kt][:, :],
                            start=(kt == 0), stop=(kt == 3))
                    ev = pw.tile([128, TCW], BF, tag="ev", bufs=2)
                    nc.vector.tensor_copy(ev[:, :], po[:, :])
                    nc.sync.dma_start(out_ap[otc, dt], ev[:, :])
                return emit
            return [dt_chunk(dt) for dt in range(NKD)]

        def attn_iter(s, p, kj, qall, augA, augB, last_kj):
            c0 = 128 * max(0, kj - 4 * s)          # causal column restriction
            n = TCW - c0
            qs0 = p * TCW + c0
            ks = slice(kj * 128, (kj + 1) * 128)
            scp = psc.tile([128, 2 * TCW], F32, tag="scp")
            nc.tensor.matmul(scp[:, c0:TCW], kpack[0:64, ks], qall[0:64, qs0:(p + 1) * TCW],
                             start=True, stop=True, tile_position=(0, 0))
            nc.tensor.matmul(scp[:, TCW + c0:2 * TCW], kpack[64:128, ks], qall[64:128, qs0:(p + 1) * TCW],
                             start=True, stop=True, tile_position=(64, 0))
            ep = pw.tile([128, 2 * TCW], BF, tag="ep", bufs=5)
            s3 = scp[:, :].rearrange("p (a b) -> p a b", a=2)[:, :, c0:TCW]
            e3 = ep[:, :].rearrange("p (a b) -> p a b", a=2)[:, :, c0:TCW]
            nc.scalar.activation(e3, s3, AF.Exp, scale=SCALE)
            dj = kj - 4 * s
            if dj >= 0:
                mb = (msk_t[:, dj * TCW + c0:(dj + 1) * TCW].unsqueeze(1)
                      .broadcast_to([128, 2, n]))
                with tcx.high_priority():
                    nc.vector.tensor_mul(e3, e3, mb)
            colA = (0 * NKJ + kj) * 65
            colB = (1 * NKJ + kj) * 65
            nc.tensor.matmul(augA[:, c0:TCW], vaug[:, colA:colA + 65], ep[:, c0:TCW],
                             start=(kj == 0), stop=(kj == last_kj))
            nc.tensor.matmul(augB[:, c0:TCW], vaug[:, colB:colB + 65], ep[:, TCW + c0:2 * TCW],
                             start=(kj == 0), stop=(kj == last_kj))
            if dbg and s == 0 and p == 0 and kj == 0:
                nc.sync.dma_start(dbg_ep[:, :], ep[:, :])

        def pack_end(s, p, augA, augB):
            """Evacuate aug psum, normalize (DVE recip), build the pair tile."""
            augSA = pw.tile([64, TCW], F32, tag="augSA", bufs=4)
            augSB = pw.tile([64, TCW], F32, tag="augSB", bufs=4)
            dnA = pw.tile([1, TCW], F32, tag="dn", bufs=9)
            dnB = pw.tile([1, TCW], F32, tag="dn", bufs=9)
            with tcx.high_priority():
                nc.vector.tensor_copy(augSA[:, :], augA[0:64, :])
                nc.vector.tensor_copy(dnA[:, :], augA[64:65, :])
                nc.vector.tensor_copy(augSB[:, :], augB[0:64, :])
                nc.vector.tensor_copy(dnB[:, :], augB[64:65, :])
            rcA = pw.tile([1, TCW], F32, tag="rc", bufs=9)
            rcB = pw.tile([1, TCW], F32, tag="rc", bufs=9)
            nc.vector.reciprocal_approx_fast(rcA[:, :], dnA[:, :])
            nc.vector.reciprocal_approx_fast(rcB[:, :], dnB[:, :])
            pair = pw.tile([128, TCW], BF, tag="pair", bufs=8)
            bA = pw.tile([64, TCW], F32, tag="bA", bufs=2)
            nc.gpsimd.partition_broadcast(bA[:, :], rcA[:, :])
            nc.vector.tensor_mul(pair[0:64, :], augSA[0:64, :], bA[:, :])
            bB = pw.tile([64, TCW], F32, tag="bB", bufs=2)
            nc.gpsimd.partition_broadcast(bB[:, :], rcB[:, :])
            ob = pw.tile([64, TCW], BF, tag="ob", bufs=3)
            nc.vector.tensor_mul(ob[:, :], augSB[0:64, :], bB[:, :])
            nc.sync.dma_start(pair[64:128, :], ob[:, :])
            if dbg and s == 0 and p == 0:
                nc.sync.dma_start(dbg_aug[0:64, :], augSA[:, :])
                nc.sync.dma_start(dbg_aug[64:65, :], dnA[:, :])
                nc.sync.dma_start(dbg_pair[0:64, :], pair[0:64, :])
                nc.sync.dma_start(dbg_pair[64:128, :], ob[:, :])
            _pairs[p] = pair

        # ---- prologue: qkv + rope for chunk 0 ----
        for c in qkv_chunks(0):
            c()
        if dbg:
            nc.sync.dma_start(dbg_q0[:, :], qropes[0][:, :])
            nc.sync.dma_start(dbg_k[:, 0:TCW], kpack[:, 0:TCW])

        # ---- main slots ----
        for s in range(TC):
            work = []
            if s + 1 < TC:
                work += qkv_chunks(s + 1)
            if s >= 1:
                work += oproj_chunks(s - 1, prev_pairs[0])
            qall = qropes[s]
            last_kj = 4 * s + 3
            n_iters = 4 * (4 * s + 4)
            W = len(work)
            ai = 0
            wi = 0
            for p in range(4):
                augA = paug.tile([65, TCW], F32, tag="augA")
                augB = paug.tile([65, TCW], F32, tag="augB")
                for kj in range(4 * s + 4):
                    attn_iter(s, p, kj, qall, augA, augB, last_kj)
                    ai += 1
                    while wi < W and wi * n_iters < ai * W:
                        work[wi]()
                        wi += 1
                pack_end(s, p, augA, augB)
            while wi < W:
                work[wi]()
                wi += 1
            prev_pairs[0] = list(_pairs)

        # ---- epilogue: o_proj of the last chunk ----
        for c in oproj_chunks(TC - 1, prev_pairs[0]):
            c()
        if dbg:
            nc.sync.dma_start(dbg_v[:, :], vaug[:, :])

    nc.compile()
    return nc


def _host_prep(x, Wqkv, Wo):
    """Build per-core input maps. Returns list of 8 dicts."""
    # RoPE row permutation: within each 64-row head, reorder rows so the
    # half-split pair (r, r+32) lands at distance 16 inside one 32-block;
    # the on-device swap is then one full-width stream_shuffle.
    perm64 = np.r_[0:16, 32:48, 16:32, 48:64]

    invfreq = 1.0 / (ROPE_BASE ** (np.arange(0, HD, 2, dtype=np.float32) / HD))
    ang = np.arange(L, dtype=np.float32)[:, None] * invfreq[None, :]   # [L, 32]
    cos = np.cos(ang).T     # [32, L]
    sin = np.sin(ang).T
    cc = np.tile(cos, (4, 1)).astype(np.float32)                       # [128, L]
    sgn = np.repeat(np.array([-1.0, 1.0, -1.0, 1.0], np.float32), 32)
    ss = (np.tile(sin, (4, 1)) * sgn[:, None]).astype(np.float32)
    perm128 = np.concatenate([perm64, perm64 + 64])
    ccss = np.concatenate([cc[perm128], ss[perm128]], axis=1).astype(BF16)

    r = np.arange(128)[:, None]
    c = np.arange(TCW)[None, :]
    msk = np.concatenate(
        [(r + 128 * j <= c).astype(np.float32) for j in range(4)], axis=1
    ).astype(BF16)                                                     # [128, 2048]

    id2 = np.zeros((128, 64), np.float32)
    id2[:64] = np.eye(64, dtype=np.float32)
    id2[64:] = np.eye(64, dtype=np.float32)
    id2 = id2.astype(BF16)

    wq_part = Wqkv[:NH * HD].reshape(NH, HD, D)
    wk_part = Wqkv[NH * HD:NH * HD + NKV * HD].reshape(NKV, HD, D)
    wv_part = Wqkv[NH * HD + NKV * HD:].reshape(NKV, HD, D)

    in_maps = []
    for core in range(8):
        b, g = core // 4, core % 4
        xT = np.ascontiguousarray(x[b].T)                              # [D, L]
        xh = (xT.reshape(NKD, 128, TC, TCW).transpose(2, 1, 0, 3)
              .reshape(TC, 128, NKD * TCW)).astype(BF16)

        rows = []
        for p in range(4):
            rows.append(wq_part[8 * g + p][perm64])
            rows.append(wq_part[8 * g + 4 + p][perm64])
        rows.append(wk_part[2 * g][perm64]); rows.append(wk_part[2 * g + 1][perm64])
        rows.append(wv_part[2 * g]); rows.append(wv_part[2 * g + 1])
        Wc = np.concatenate(rows, axis=0)                              # [768, D]
        wq = (Wc.reshape(NOT, 128, NKD, 128).transpose(3, 2, 0, 1)
              .reshape(128, NKD * NOT * 128)).astype(BF16)

        cols = np.empty((4, 128), np.int64)
        for kt in range(4):
            cols[kt, :64] = (8 * g + kt) * HD + np.arange(64)
            cols[kt, 64:] = (8 * g + 4 + kt) * HD + np.arange(64)
        Woc = Wo.T[cols.reshape(-1)]                                   # [512, D]
        wo = (Woc.reshape(4, 128, NKD, 128).transpose(1, 0, 2, 3)
              .reshape(128, 4 * NKD * 128)).astype(BF16)

        in_maps.append(dict(xh=xh, wq=wq, wo=wo, ccss=ccss, msk=msk, id2=id2))
    return in_maps


def _get_nc():
    if "nc" not in _cached:
        _cached["nc"] = _build_nc()
    return _cached["nc"]


def run_sharded(x, Wqkv, Wo, trace=False):
    """Run on 8 cores; returns (out [B,L,D] float32, BassKernelResults)."""
    from concourse.bass_utils import run_bass_kernel_spmd
    nc = _get_nc()
    in_maps = _host_prep(np.asarray(x, np.float32), np.asarray(Wqkv, np.float32),
                         np.asarray(Wo, np.float32))
    res = run_bass_kernel_spmd(nc, in_maps, list(range(8)), trace=trace)
    out = np.zeros((B, L, D), np.float32)
    for core in range(8):
        b = core // 4
        P = res.results[core]["outp"].astype(np.float32).transpose(1, 2, 0, 3).reshape(D, L)
        out[b] += P.T
    return out, res


def kernel(x, Wqkv, Wo):
    out, _ = run_sharded(x, Wqkv, Wo, trace=False)
    return out


# revision 12
# speedup vs baseline: 1.0192x; 1.0192x over previous
"""TP(heads)xDP(batch) sharded causal GQA attention block for 8 trn2 cores.

Each core c handles batch b=c//4 and head group g=c%4 (8 q heads, 2 kv heads).
v2 pipeline (software-pipelined one chunk ahead, PE-stream interleaved):
  slot s emits: attention(s) iterations interleaved with qkv(s+1) matmul
  chunks and o_proj(s-1) tiles, so the PE never waits on RoPE/exp/norm.
  - RoPE rows are pre-permuted on the host so the half-swap is a single
    full-width (128-partition) stream_shuffle; all rope math in bf16.
  - scores_T[kj, qi] = k^T q  (bf16, 2 heads packed via PE row tiling)
  - exp on ScalarE (no max subtraction; scores ~N(0,1) after 1/8 scale),
    diagonal blocks N-restricted to the causal region.
  - out_aug[65, qi] = [v; ones]^T exp  (bf16; row 64 = softmax denominator)
  - normalize via DVE reciprocal_approx_fast + gpsimd partition_broadcast
  - partial_T[dout, t] = Wo_c^T attn (bf16 out); host sums the 8 partials.
"""
import sys
sys.path.insert(0, "/opt/trn_rl_repo")
from contextlib import ExitStack

import numpy as np
import ml_dtypes

B, L, D = 2, 2048, 2048
NH, NKV, HD = 32, 8, 64
ROPE_BASE = 10000.0
SCALE = HD ** -0.5
TC, TCW = 4, 512      # query/token chunks
NKD = 16              # d contraction tiles
NOT = 6               # output tiles per core (4 q packs, k pack, v pack)
NKJ = 16              # key tiles

BF16 = ml_dtypes.bfloat16

_cached = {}

# swap halves of 16 within each 32-partition quadrant (rope pair swap after
# the host-side row permutation)
SWAP16 = [(i + 16) % 32 for i in range(32)]


def _build_nc(dbg=False):
    import concourse.bacc as bacc
    import concourse.tile as tile
    import concourse.mybir as mybir
    from concourse import library_config

    F32 = mybir.dt.float32
    BF = mybir.dt.bfloat16
    AF = mybir.ActivationFunctionType

    nc = bacc.Bacc("TRN2", debug=False)
    xh_ap = nc.dram_tensor("xh", (TC, 128, NKD * TCW), BF, kind="ExternalInput").ap()
    wq_ap = nc.dram_tensor("wq", (128, NKD * NOT * 128), BF, kind="ExternalInput").ap()
    wo_ap = nc.dram_tensor("wo", (128, 4 * NKD * 128), BF, kind="ExternalInput").ap()
    ccss_ap = nc.dram_tensor("ccss", (128, 2 * L), BF, kind="ExternalInput").ap()
    msk_ap = nc.dram_tensor("msk", (128, 4 * TCW), BF, kind="ExternalInput").ap()
    id2_ap = nc.dram_tensor("id2", (128, 64), BF, kind="ExternalInput").ap()
    out_ap = nc.dram_tensor("outp", (TC, NKD, 128, TCW), BF, kind="ExternalOutput").ap()
    if dbg:
        dbg_q0 = nc.dram_tensor("dbg_q0", (128, 4 * TCW), BF, kind="ExternalOutput").ap()
        dbg_k = nc.dram_tensor("dbg_k", (128, L), BF, kind="ExternalOutput").ap()
        dbg_v = nc.dram_tensor("dbg_v", (128, 2 * NKJ * 65), BF, kind="ExternalOutput").ap()
        dbg_ep = nc.dram_tensor("dbg_ep", (128, 2 * TCW), BF, kind="ExternalOutput").ap()
        dbg_aug = nc.dram_tensor("dbg_aug", (65, TCW), F32, kind="ExternalOutput").ap()
        dbg_pair = nc.dram_tensor("dbg_pair", (128, TCW), BF, kind="ExternalOutput").ap()

    with tile.TileContext(nc) as tcx, ExitStack() as ctx:
        pc = ctx.enter_context(tcx.tile_pool(name="const", bufs=1))
        px = ctx.enter_context(tcx.tile_pool(name="x", bufs=2))
        pw = ctx.enter_context(tcx.tile_pool(name="work", bufs=1))
        psc = ctx.enter_context(tcx.tile_pool(name="psc", bufs=2, space="PSUM"))
        paug = ctx.enter_context(tcx.tile_pool(name="paug", bufs=1, space="PSUM"))
        pmm = ctx.enter_context(tcx.tile_pool(name="pmm", bufs=2, space="PSUM"))

        wq_t = pc.tile([128, NKD * NOT * 128], BF)
        wo_t = pc.tile([128, 4 * NKD * 128], BF)
        ccss_t = pc.tile([128, 2 * L], BF)
        msk_t = pc.tile([128, 4 * TCW], BF)
        id2_t = pc.tile([128, 64], BF)
        kpack = pc.tile([128, L], BF)            # k (2 kv heads stacked), RoPE'd
        vaug = pc.tile([128, 2 * NKJ * 65], BF)  # [v | ones] per (kv, kj)
        nc.vector.memset(vaug[:, 64::65], 1.0)   # ones columns

        with tcx.tile_critical():
            nc.gpsimd.load_library(library_config.attn)

        def emit_const_dmas_early():
            # wq per-ot (strided) in the order the prologue consumes it
            wqv = wq_t[:, :].rearrange("p (d o r) -> p d o r", d=NKD, o=NOT)
            wqs = wq_ap[:, :].rearrange("p (d o r) -> p d o r", d=NKD, o=NOT)
            for ot in (4, 5, 0, 1, 2, 3):
                nc.sync.dma_start(wqv[:, :, ot, :], wqs[:, :, ot, :])
            # rope tables for chunk 0 only; rest comes later
            nc.sync.dma_start(ccss_t[:, 0:TCW], ccss_ap[:, 0:TCW])
            nc.sync.dma_start(ccss_t[:, L:L + TCW], ccss_ap[:, L:L + TCW])
            nc.sync.dma_start(id2_t[:, :], id2_ap[:, :])

        def emit_const_dmas_late():
            nc.sync.dma_start(msk_t[:, :], msk_ap[:, :])
            nc.sync.dma_start(ccss_t[:, TCW:L], ccss_ap[:, TCW:L])
            nc.sync.dma_start(ccss_t[:, L + TCW:2 * L], ccss_ap[:, L + TCW:2 * L])
            wow = 4 * NKD * 128 // 4
            for i in range(4):
                nc.sync.dma_start(wo_t[:, i * wow:(i + 1) * wow], wo_ap[:, i * wow:(i + 1) * wow])

        qropes = [None] * TC   # per-chunk roped q tiles
        _pairs = [None] * 4
        prev_pairs = [None]    # pairs of chunk s-1, consumed by o_proj in slot s

        def emit_x_dma(tci):
            xt = px.tile([128, NKD * TCW], BF, tag="xt")
            xw = NKD * TCW // 8
            with tcx.high_priority():
                for i in range(8):
                    nc.sync.dma_start(xt[:, i * xw:(i + 1) * xw],
                                      xh_ap[tci][:, i * xw:(i + 1) * xw])
            return xt

        def rope(dest_ap, raw, tci, nrep, w):
            """dest = raw*CC + swap16(raw)*SS over [128, w]; bf16 throughout."""
            with tcx.high_priority():
                swp = pw.tile([128, 4 * TCW], BF, tag="swp", bufs=2)
                nc.vector.stream_shuffle(swp[:, 0:w], raw[:, 0:w], SWAP16)
                cs = ccss_t[:, tci * TCW:(tci + 1) * TCW].unsqueeze(1).broadcast_to([128, nrep, TCW])
                sss = ccss_t[:, L + tci * TCW:L + (tci + 1) * TCW].unsqueeze(1).broadcast_to([128, nrep, TCW])
                r3 = raw[:, 0:w].rearrange("p (a b) -> p a b", a=nrep)
                s3 = swp[:, 0:w].rearrange("p (a b) -> p a b", a=nrep)
                nc.vector.tensor_mul(r3, r3, cs)
                nc.vector.tensor_mul(s3, s3, sss)
                nc.vector.tensor_add(dest_ap.rearrange("p (a b) -> p a b", a=nrep), r3, s3)

        def qkv_chunks(tci):
            """Return list of closures emitting the qkv projection for chunk
            tci: per-ot matmul groups (k, v first), then the q rope."""
            xt = emit_x_dma(tci)
            qraw = pw.tile([128, 4 * TCW], BF, tag="qraw", bufs=2)
            chunks = []

            def ot_chunk(ot):
                def emit():
                    ps = pmm.tile([128, TCW], F32, tag="mm")
                    for dt in range(NKD):
                        nc.tensor.matmul(
                            ps[:, :], wq_t[:, (dt * NOT + ot) * 128:(dt * NOT + ot + 1) * 128],
                            xt[:, dt * TCW:(dt + 1) * TCW],
                            start=(dt == 0), stop=(dt == NKD - 1))
                    if ot == 4:
                        kraw = pw.tile([128, TCW], BF, tag="kraw", bufs=2)
                        with tcx.high_priority():
                            nc.vector.tensor_copy(kraw[:, :], ps[:, :])
                        rope(kpack[:, tci * TCW:(tci + 1) * TCW], kraw, tci, 1, TCW)
                    elif ot == 5:
                        vch = pw.tile([128, TCW], BF, tag="vch", bufs=2)
                        with tcx.high_priority():
                            nc.vector.tensor_copy(vch[:, :], ps[:, :])
                        for j in range(2):
                            for jj in range(4):
                                kj = 4 * tci + jj
                                tp = pmm.tile([128, 64], BF, tag="mm")
                                nc.tensor.transpose(
                                    tp[:, :], vch[64 * j:64 * j + 64, jj * 128:(jj + 1) * 128],
                                    id2_t[64 * j:64 * j + 64, :])
                                col = (j * NKJ + kj) * 65
                                with tcx.high_priority():
                                    nc.vector.tensor_copy(vaug[:, col:col + 64], tp[:, :])
                    else:
                        with tcx.high_priority():
                            nc.vector.tensor_copy(qraw[:, ot * TCW:(ot + 1) * TCW], ps[:, :])
                return emit

            for ot in (4, 5, 0, 1, 2, 3):
                chunks.append(ot_chunk(ot))
            chunks.append(lambda: rope(qraw[:, :], qraw, tci, 4, 4 * TCW))
            qropes[tci] = qraw
            return chunks

        def oproj_chunks(otc, pairs_):
            """16 closures, one per dout tile (4 matmuls each). Evacuation
            alternates DVE/ScalarE so psum banks recycle twice as fast."""
            def dt_chunk(dt):
                def emit():
                    po = pmm.tile([128, TCW], F32, tag="mm")
                    for kt in range(4):
                        nc.tensor.matmul(
                            po[:, :], wo_t[:, (kt * NKD + dt) * 128:(kt * NKD + dt + 1) * 128],
                            pairs_[kt][:, :],
                            start=(kt == 0), stop=(kt == 3))
                    ev = pw.tile([128, TCW], BF, tag="ev", bufs=3)
                    if dt % 2:
                        nc.scalar.copy(ev[:, :], po[:, :])
                    else:
                        nc.vector.tensor_copy(ev[:, :], po[:, :])
                    nc.sync.dma_start(out_ap[otc, dt], ev[:, :])
                return emit
            return [dt_chunk(dt) for dt in range(NKD)]

        def attn_iter(s, p, kj, qall, augA, augB, last_kj):
            c0 = 128 * max(0, kj - 4 * s)          # causal column restriction
            n = TCW - c0
            qs0 = p * TCW + c0
            ks = slice(kj * 128, (kj + 1) * 128)
            scp = psc.tile([128, 2 * TCW], F32, tag="scp")
            nc.tensor.matmul(scp[:, c0:TCW], kpack[0:64, ks], qall[0:64, qs0:(p + 1) * TCW],
                             start=True, stop=True, tile_position=(0, 0))
            nc.tensor.matmul(scp[:, TCW + c0:2 * TCW], kpack[64:128, ks], qall[64:128, qs0:(p + 1) * TCW],
                             start=True, stop=True, tile_position=(64, 0))
            ep = pw.tile([128, 2 * TCW], BF, tag="ep", bufs=5)
            s3 = scp[:, :].rearrange("p (a b) -> p a b", a=2)[:, :, c0:TCW]
            e3 = ep[:, :].rearrange("p (a b) -> p a b", a=2)[:, :, c0:TCW]
            nc.scalar.activation(e3, s3, AF.Exp, scale=SCALE)
            dj = kj - 4 * s
            if dj >= 0:
                mb = (msk_t[:, dj * TCW + c0:(dj + 1) * TCW].unsqueeze(1)
                      .broadcast_to([128, 2, n]))
                with tcx.high_priority():
                    nc.vector.tensor_mul(e3, e3, mb)
            colA = (0 * NKJ + kj) * 65
            colB = (1 * NKJ + kj) * 65
            nc.tensor.matmul(augA[:, c0:TCW], vaug[:, colA:colA + 65], ep[:, c0:TCW],
                             start=(kj == 0), stop=(kj == last_kj))
            nc.tensor.matmul(augB[:, c0:TCW], vaug[:, colB:colB + 65], ep[:, TCW + c0:2 * TCW],
                             start=(kj == 0), stop=(kj == last_kj))
            if dbg and s == 0 and p == 0 and kj == 0:
                nc.sync.dma_start(dbg_ep[:, :], ep[:, :])

        def pack_end(s, p, augA, augB):
            """Evacuate aug psum, normalize (DVE recip), build the pair tile."""
            augSA = pw.tile([64, TCW], F32, tag="augSA", bufs=4)
            augSB = pw.tile([64, TCW], F32, tag="augSB", bufs=4)
            dnA = pw.tile([1, TCW], F32, tag="dn", bufs=9)
            dnB = pw.tile([1, TCW], F32, tag="dn", bufs=9)
            with tcx.high_priority():
                nc.vector.tensor_copy(augSA[:, :], augA[0:64, :])
                nc.vector.tensor_copy(dnA[:, :], augA[64:65, :])
                nc.vector.tensor_copy(augSB[:, :], augB[0:64, :])
                nc.vector.tensor_copy(dnB[:, :], augB[64:65, :])
            rcA = pw.tile([1, TCW], F32, tag="rc", bufs=9)
            rcB = pw.tile([1, TCW], F32, tag="rc", bufs=9)
            nc.vector.reciprocal_approx_fast(rcA[:, :], dnA[:, :])
            nc.vector.reciprocal_approx_fast(rcB[:, :], dnB[:, :])
            pair = pw.tile([128, TCW], BF, tag="pair", bufs=8)
            bA = pw.tile([64, TCW], F32, tag="bA", bufs=2)
            nc.gpsimd.partition_broadcast(bA[:, :], rcA[:, :])
            nc.vector.tensor_mul(pair[0:64, :], augSA[0:64, :], bA[:, :])
            bB = pw.tile([64, TCW], F32, tag="bB", bufs=2)
            nc.gpsimd.partition_broadcast(bB[:, :], rcB[:, :])
            nc.vector.tensor_mul(pair[64:128, :], augSB[0:64, :], bB[:, :])
            if dbg and s == 0 and p == 0:
                nc.sync.dma_start(dbg_aug[0:64, :], augSA[:, :])
                nc.sync.dma_start(dbg_aug[64:65, :], dnA[:, :])
                nc.sync.dma_start(dbg_pair[:, :], pair[:, :])
            _pairs[p] = pair

        # ---- prologue: qkv + rope for chunk 0 ----
        chunks0 = qkv_chunks(0)      # emits the x(0) DMA first
        emit_const_dmas_early()
        for c in chunks0:
            c()
        emit_const_dmas_late()
        if dbg:
            nc.sync.dma_start(dbg_q0[:, :], qropes[0][:, :])
            nc.sync.dma_start(dbg_k[:, 0:TCW], kpack[:, 0:TCW])

        # ---- main slots ----
        for s in range(TC):
            work = []
            if s + 1 < TC:
                work += qkv_chunks(s + 1)
            if s >= 1:
                work += oproj_chunks(s - 1, prev_pairs[0])
            qall = qropes[s]
            last_kj = 4 * s + 3
            n_iters = 4 * (4 * s + 4)
            W = len(work)
            ai = 0
            wi = 0

            def drain(quota):
                nonlocal wi
                while wi < W and wi < quota:
                    work[wi]()
                    wi += 1
            drain(2)   # prime: PE filler ahead of the first (possibly waiting) scores
            for p in range(4):
                augA = paug.tile([65, TCW], F32, tag="augA")
                augB = paug.tile([65, TCW], F32, tag="augB")
                for kj in range(4 * s + 4):
                    attn_iter(s, p, kj, qall, augA, augB, last_kj)
                    ai += 1
                    drain(2 + ai * W / n_iters)
                pack_end(s, p, augA, augB)
            drain(W)
            prev_pairs[0] = list(_pairs)

        # ---- epilogue: o_proj of the last chunk ----
        for c in oproj_chunks(TC - 1, prev_pairs[0]):
            c()
        if dbg:
            nc.sync.dma_start(dbg_v[:, :], vaug[:, :])

    nc.compile()
    return nc


def _host_prep(x, Wqkv, Wo):
    """Build per-core input maps. Returns list of 8 dicts."""
    # RoPE row permutation: within each 64-row head, reorder rows so the
    # half-split pair (r, r+32) lands at distance 16 inside one 32-block;
    # the on-device swap is then one full-width stream_shuffle.
    perm64 = np.r_[0:16, 32:48, 16:32, 48:64]

    invfreq = 1.0 / (ROPE_BASE ** (np.arange(0, HD, 2, dtype=np.float32) / HD))
    ang = np.arange(L, dtype=np.float32)[:, None] * invfreq[None, :]   # [L, 32]
    cos = np.cos(ang).T     # [32, L]
    sin = np.sin(ang).T
    cc = np.tile(cos, (4, 1)).astype(np.float32)                       # [128, L]
    sgn = np.repeat(np.array([-1.0, 1.0, -1.0, 1.0], np.float32), 32)
    ss = (np.tile(sin, (4, 1)) * sgn[:, None]).astype(np.float32)
    perm128 = np.concatenate([perm64, perm64 + 64])
    ccss = np.concatenate([cc[perm128], ss[perm128]], axis=1).astype(BF16)

    r = np.arange(128)[:, None]
    c = np.arange(TCW)[None, :]
    msk = np.concatenate(
        [(r + 128 * j <= c).astype(np.float32) for j in range(4)], axis=1
    ).astype(BF16)                                                     # [128, 2048]

    id2 = np.zeros((128, 64), np.float32)
    id2[:64] = np.eye(64, dtype=np.float32)
    id2[64:] = np.eye(64, dtype=np.float32)
    id2 = id2.astype(BF16)

    wq_part = Wqkv[:NH * HD].reshape(NH, HD, D)
    wk_part = Wqkv[NH * HD:NH * HD + NKV * HD].reshape(NKV, HD, D)
    wv_part = Wqkv[NH * HD + NKV * HD:].reshape(NKV, HD, D)

    in_maps = []
    for core in range(8):
        b, g = core // 4, core % 4
        xT = np.ascontiguousarray(x[b].T)                              # [D, L]
        xh = (xT.reshape(NKD, 128, TC, TCW).transpose(2, 1, 0, 3)
              .reshape(TC, 128, NKD * TCW)).astype(BF16)

        rows = []
        for p in range(4):
            rows.append(wq_part[8 * g + p][perm64])
            rows.append(wq_part[8 * g + 4 + p][perm64])
        rows.append(wk_part[2 * g][perm64]); rows.append(wk_part[2 * g + 1][perm64])
        rows.append(wv_part[2 * g]); rows.append(wv_part[2 * g + 1])
        Wc = np.concatenate(rows, axis=0)                              # [768, D]
        wq = (Wc.reshape(NOT, 128, NKD, 128).transpose(3, 2, 0, 1)
              .reshape(128, NKD * NOT * 128)).astype(BF16)

        cols = np.empty((4, 128), np.int64)
        for kt in range(4):
            cols[kt, :64] = (8 * g + kt) * HD + np.arange(64)
            cols[kt, 64:] = (8 * g + 4 + kt) * HD + np.arange(64)
        Woc = Wo.T[cols.reshape(-1)]                                   # [512, D]
        wo = (Woc.reshape(4, 128, NKD, 128).transpose(1, 0, 2, 3)
              .reshape(128, 4 * NKD * 128)).astype(BF16)

        in_maps.append(dict(xh=xh, wq=wq, wo=wo, ccss=ccss, msk=msk, id2=id2))
    return in_maps


def _get_nc():
    if "nc" not in _cached:
        _cached["nc"] = _build_nc()
    return _cached["nc"]


def run_sharded(x, Wqkv, Wo, trace=False):
    """Run on 8 cores; returns (out [B,L,D] float32, BassKernelResults)."""
    from concourse.bass_utils import run_bass_kernel_spmd
    nc = _get_nc()
    in_maps = _host_prep(np.asarray(x, np.float32), np.asarray(Wqkv, np.float32),
                         np.asarray(Wo, np.float32))
    res = run_bass_kernel_spmd(nc, in_maps, list(range(8)), trace=trace)
    out = np.zeros((B, L, D), np.float32)
    for core in range(8):
        b = core // 4
        P = res.results[core]["outp"].astype(np.float32).transpose(1, 2, 0, 3).reshape(D, L)
        out[b] += P.T
    return out, res


def kernel(x, Wqkv, Wo):
    out, _ = run_sharded(x, Wqkv, Wo, trace=False)
    return out


# revision 13
# speedup vs baseline: 1.0253x; 1.0060x over previous
"""TP(heads)xDP(batch) sharded causal GQA attention block for 8 trn2 cores.

Each core c handles batch b=c//4 and head group g=c%4 (8 q heads, 2 kv heads).
v2 pipeline (software-pipelined one chunk ahead, PE-stream interleaved):
  slot s emits: attention(s) iterations interleaved with qkv(s+1) matmul
  chunks and o_proj(s-1) tiles, so the PE never waits on RoPE/exp/norm.
  - RoPE rows are pre-permuted on the host so the half-swap is a single
    full-width (128-partition) stream_shuffle; all rope math in bf16.
  - scores_T[kj, qi] = k^T q  (bf16, 2 heads packed via PE row tiling)
  - exp on ScalarE (no max subtraction; scores ~N(0,1) after 1/8 scale),
    diagonal blocks N-restricted to the causal region.
  - out_aug[65, qi] = [v; ones]^T exp  (bf16; row 64 = softmax denominator)
  - normalize via DVE reciprocal_approx_fast + gpsimd partition_broadcast
  - partial_T[dout, t] = Wo_c^T attn (bf16 out); host sums the 8 partials.
"""
import sys
sys.path.insert(0, "/opt/trn_rl_repo")
from contextlib import ExitStack

import numpy as np
import ml_dtypes

B, L, D = 2, 2048, 2048
NH, NKV, HD = 32, 8, 64
ROPE_BASE = 10000.0
SCALE = HD ** -0.5
TC, TCW = 4, 512      # query/token chunks
NKD = 16              # d contraction tiles
NOT = 6               # output tiles per core (4 q packs, k pack, v pack)
NKJ = 16              # key tiles

BF16 = ml_dtypes.bfloat16

_cached = {}

# swap halves of 16 within each 32-partition quadrant (rope pair swap after
# the host-side row permutation)
SWAP16 = [(i + 16) % 32 for i in range(32)]


def _build_nc(dbg=False):
    import concourse.bacc as bacc
    import concourse.tile as tile
    import concourse.mybir as mybir
    from concourse import library_config

    F32 = mybir.dt.float32
    BF = mybir.dt.bfloat16
    AF = mybir.ActivationFunctionType

    nc = bacc.Bacc("TRN2", debug=False)
    xh_ap = nc.dram_tensor("xh", (TC, 128, NKD * TCW), BF, kind="ExternalInput").ap()
    wq_ap = nc.dram_tensor("wq", (128, NKD * NOT * 128), BF, kind="ExternalInput").ap()
    wo_ap = nc.dram_tensor("wo", (128, 4 * NKD * 128), BF, kind="ExternalInput").ap()
    ccss_ap = nc.dram_tensor("ccss", (128, 2 * L), BF, kind="ExternalInput").ap()
    msk_ap = nc.dram_tensor("msk", (128, 4 * TCW), BF, kind="ExternalInput").ap()
    id2_ap = nc.dram_tensor("id2", (128, 64), BF, kind="ExternalInput").ap()
    out_ap = nc.dram_tensor("outp", (TC, NKD, 128, TCW), BF, kind="ExternalOutput").ap()
    if dbg:
        dbg_q0 = nc.dram_tensor("dbg_q0", (128, 4 * TCW), BF, kind="ExternalOutput").ap()
        dbg_k = nc.dram_tensor("dbg_k", (128, L), BF, kind="ExternalOutput").ap()
        dbg_v = nc.dram_tensor("dbg_v", (128, 2 * NKJ * 65), BF, kind="ExternalOutput").ap()
        dbg_ep = nc.dram_tensor("dbg_ep", (128, 2 * TCW), BF, kind="ExternalOutput").ap()
        dbg_aug = nc.dram_tensor("dbg_aug", (65, TCW), F32, kind="ExternalOutput").ap()
        dbg_pair = nc.dram_tensor("dbg_pair", (128, TCW), BF, kind="ExternalOutput").ap()

    with tile.TileContext(nc) as tcx, ExitStack() as ctx:
        pc = ctx.enter_context(tcx.tile_pool(name="const", bufs=1))
        px = ctx.enter_context(tcx.tile_pool(name="x", bufs=2))
        pw = ctx.enter_context(tcx.tile_pool(name="work", bufs=1))
        psc = ctx.enter_context(tcx.tile_pool(name="psc", bufs=2, space="PSUM"))
        paug = ctx.enter_context(tcx.tile_pool(name="paug", bufs=1, space="PSUM"))
        pmm = ctx.enter_context(tcx.tile_pool(name="pmm", bufs=2, space="PSUM"))

        wq_t = pc.tile([128, NKD * NOT * 128], BF)
        wo_t = pc.tile([128, 4 * NKD * 128], BF)
        ccss_t = pc.tile([128, 2 * L], BF)
        msk_t = pc.tile([128, 4 * TCW], BF)
        id2_t = pc.tile([128, 64], BF)
        kpack = pc.tile([128, L], BF)            # k (2 kv heads stacked), RoPE'd
        vaug = pc.tile([128, 2 * NKJ * 65], BF)  # [v | ones] per (kv, kj)
        nc.vector.memset(vaug[:, 64::65], 1.0)   # ones columns

        with tcx.tile_critical():
            nc.gpsimd.load_library(library_config.attn)

        def emit_const_dmas_early():
            # contiguous wq pieces (dt-major): MM(dt) unblocks as piece dt//2 lands
            wqw = NKD * NOT * 128 // 8
            for i in range(8):
                nc.sync.dma_start(wq_t[:, i * wqw:(i + 1) * wqw],
                                  wq_ap[:, i * wqw:(i + 1) * wqw])
            # rope tables for chunk 0 only; rest comes later
            nc.sync.dma_start(ccss_t[:, 0:TCW], ccss_ap[:, 0:TCW])
            nc.sync.dma_start(ccss_t[:, L:L + TCW], ccss_ap[:, L:L + TCW])
            nc.sync.dma_start(id2_t[:, :], id2_ap[:, :])

        def emit_const_dmas_late():
            nc.sync.dma_start(msk_t[:, :], msk_ap[:, :])
            nc.sync.dma_start(ccss_t[:, TCW:L], ccss_ap[:, TCW:L])
            nc.sync.dma_start(ccss_t[:, L + TCW:2 * L], ccss_ap[:, L + TCW:2 * L])
            wow = 4 * NKD * 128 // 4
            for i in range(4):
                nc.sync.dma_start(wo_t[:, i * wow:(i + 1) * wow], wo_ap[:, i * wow:(i + 1) * wow])

        qropes = [None] * TC   # per-chunk roped q tiles
        _pairs = [None] * 4
        prev_pairs = [None]    # pairs of chunk s-1, consumed by o_proj in slot s

        def emit_x_dma(tci):
            xt = px.tile([128, NKD * TCW], BF, tag="xt")
            xw = NKD * TCW // 8
            with tcx.high_priority():
                for i in range(8):
                    nc.sync.dma_start(xt[:, i * xw:(i + 1) * xw],
                                      xh_ap[tci][:, i * xw:(i + 1) * xw])
            return xt

        def rope(dest_ap, raw, tci, nrep, w):
            """dest = raw*CC + swap16(raw)*SS over [128, w]; bf16 throughout."""
            with tcx.high_priority():
                swp = pw.tile([128, 4 * TCW], BF, tag="swp", bufs=2)
                nc.vector.stream_shuffle(swp[:, 0:w], raw[:, 0:w], SWAP16)
                cs = ccss_t[:, tci * TCW:(tci + 1) * TCW].unsqueeze(1).broadcast_to([128, nrep, TCW])
                sss = ccss_t[:, L + tci * TCW:L + (tci + 1) * TCW].unsqueeze(1).broadcast_to([128, nrep, TCW])
                r3 = raw[:, 0:w].rearrange("p (a b) -> p a b", a=nrep)
                s3 = swp[:, 0:w].rearrange("p (a b) -> p a b", a=nrep)
                nc.vector.tensor_mul(r3, r3, cs)
                nc.vector.tensor_mul(s3, s3, sss)
                nc.vector.tensor_add(dest_ap.rearrange("p (a b) -> p a b", a=nrep), r3, s3)

        def qkv_chunks(tci):
            """Return list of closures emitting the qkv projection for chunk
            tci: per-ot matmul groups (k, v first), then the q rope."""
            xt = emit_x_dma(tci)
            qraw = pw.tile([128, 4 * TCW], BF, tag="qraw", bufs=2)
            chunks = []

            def ot_chunk(ot):
                def emit():
                    ps = pmm.tile([128, TCW], F32, tag="mm")
                    for dt in range(NKD):
                        nc.tensor.matmul(
                            ps[:, :], wq_t[:, (dt * NOT + ot) * 128:(dt * NOT + ot + 1) * 128],
                            xt[:, dt * TCW:(dt + 1) * TCW],
                            start=(dt == 0), stop=(dt == NKD - 1))
                    if ot == 4:
                        kraw = pw.tile([128, TCW], BF, tag="kraw", bufs=2)
                        with tcx.high_priority():
                            nc.vector.tensor_copy(kraw[:, :], ps[:, :])
                        rope(kpack[:, tci * TCW:(tci + 1) * TCW], kraw, tci, 1, TCW)
                    elif ot == 5:
                        vch = pw.tile([128, TCW], BF, tag="vch", bufs=2)
                        with tcx.high_priority():
                            nc.vector.tensor_copy(vch[:, :], ps[:, :])
                        for j in range(2):
                            for jj in range(4):
                                kj = 4 * tci + jj
                                tp = pmm.tile([128, 64], BF, tag="mm")
                                nc.tensor.transpose(
                                    tp[:, :], vch[64 * j:64 * j + 64, jj * 128:(jj + 1) * 128],
                                    id2_t[64 * j:64 * j + 64, :])
                                col = (j * NKJ + kj) * 65
                                with tcx.high_priority():
                                    nc.vector.tensor_copy(vaug[:, col:col + 64], tp[:, :])
                    else:
                        with tcx.high_priority():
                            nc.vector.tensor_copy(qraw[:, ot * TCW:(ot + 1) * TCW], ps[:, :])
                return emit

            for ot in (4, 5, 0, 1, 2, 3):
                chunks.append(ot_chunk(ot))
            chunks.append(lambda: rope(qraw[:, :], qraw, tci, 4, 4 * TCW))
            qropes[tci] = qraw
            return chunks

        def oproj_chunks(otc, pairs_):
            """16 closures, one per dout tile (4 matmuls each). Evacuation
            alternates DVE/ScalarE so psum banks recycle twice as fast."""
            def dt_chunk(dt):
                def emit():
                    po = pmm.tile([128, TCW], F32, tag="mm")
                    for kt in range(4):
                        nc.tensor.matmul(
                            po[:, :], wo_t[:, (kt * NKD + dt) * 128:(kt * NKD + dt + 1) * 128],
                            pairs_[kt][:, :],
                            start=(kt == 0), stop=(kt == 3))
                    ev = pw.tile([128, TCW], BF, tag="ev", bufs=3)
                    if dt % 2:
                        nc.scalar.copy(ev[:, :], po[:, :])
                    else:
                        nc.vector.tensor_copy(ev[:, :], po[:, :])
                    nc.sync.dma_start(out_ap[otc, dt], ev[:, :])
                return emit
            return [dt_chunk(dt) for dt in range(NKD)]

        def attn_iter(s, p, kj, qall, augA, augB, last_kj):
            c0 = 128 * max(0, kj - 4 * s)          # causal column restriction
            n = TCW - c0
            qs0 = p * TCW + c0
            ks = slice(kj * 128, (kj + 1) * 128)
            scp = psc.tile([128, 2 * TCW], F32, tag="scp")
            nc.tensor.matmul(scp[:, c0:TCW], kpack[0:64, ks], qall[0:64, qs0:(p + 1) * TCW],
                             start=True, stop=True, tile_position=(0, 0))
            nc.tensor.matmul(scp[:, TCW + c0:2 * TCW], kpack[64:128, ks], qall[64:128, qs0:(p + 1) * TCW],
                             start=True, stop=True, tile_position=(64, 0))
            ep = pw.tile([128, 2 * TCW], BF, tag="ep", bufs=5)
            s3 = scp[:, :].rearrange("p (a b) -> p a b", a=2)[:, :, c0:TCW]
            e3 = ep[:, :].rearrange("p (a b) -> p a b", a=2)[:, :, c0:TCW]
            nc.scalar.activation(e3, s3, AF.Exp, scale=SCALE)
            dj = kj - 4 * s
            if dj >= 0:
                mb = (msk_t[:, dj * TCW + c0:(dj + 1) * TCW].unsqueeze(1)
                      .broadcast_to([128, 2, n]))
                with tcx.high_priority():
                    nc.vector.tensor_mul(e3, e3, mb)
            colA = (0 * NKJ + kj) * 65
            colB = (1 * NKJ + kj) * 65
            nc.tensor.matmul(augA[:, c0:TCW], vaug[:, colA:colA + 65], ep[:, c0:TCW],
                             start=(kj == 0), stop=(kj == last_kj))
            nc.tensor.matmul(augB[:, c0:TCW], vaug[:, colB:colB + 65], ep[:, TCW + c0:2 * TCW],
                             start=(kj == 0), stop=(kj == last_kj))
            if dbg and s == 0 and p == 0 and kj == 0:
                nc.sync.dma_start(dbg_ep[:, :], ep[:, :])

        def pack_end(s, p, augA, augB):
            """Evacuate aug psum, normalize (DVE recip), build the pair tile."""
            augSA = pw.tile([64, TCW], F32, tag="augSA", bufs=4)
            augSB = pw.tile([64, TCW], F32, tag="augSB", bufs=4)
            dnA = pw.tile([1, TCW], F32, tag="dn", bufs=9)
            dnB = pw.tile([1, TCW], F32, tag="dn", bufs=9)
            with tcx.high_priority():
                nc.vector.tensor_copy(augSA[:, :], augA[0:64, :])
                nc.vector.tensor_copy(dnA[:, :], augA[64:65, :])
                nc.vector.tensor_copy(augSB[:, :], augB[0:64, :])
                nc.vector.tensor_copy(dnB[:, :], augB[64:65, :])
            rcA = pw.tile([1, TCW], F32, tag="rc", bufs=9)
            rcB = pw.tile([1, TCW], F32, tag="rc", bufs=9)
            nc.vector.reciprocal_approx_fast(rcA[:, :], dnA[:, :])
            nc.vector.reciprocal_approx_fast(rcB[:, :], dnB[:, :])
            pair = pw.tile([128, TCW], BF, tag="pair", bufs=8)
            bA = pw.tile([64, TCW], F32, tag="bA", bufs=2)
            nc.gpsimd.partition_broadcast(bA[:, :], rcA[:, :])
            nc.vector.tensor_mul(pair[0:64, :], augSA[0:64, :], bA[:, :])
            bB = pw.tile([64, TCW], F32, tag="bB", bufs=2)
            nc.gpsimd.partition_broadcast(bB[:, :], rcB[:, :])
            nc.vector.tensor_mul(pair[64:128, :], augSB[0:64, :], bB[:, :])
            if dbg and s == 0 and p == 0:
                nc.sync.dma_start(dbg_aug[0:64, :], augSA[:, :])
                nc.sync.dma_start(dbg_aug[64:65, :], dnA[:, :])
                nc.sync.dma_start(dbg_pair[:, :], pair[:, :])
            _pairs[p] = pair

        # ---- prologue: qkv + rope for chunk 0 ----
        chunks0 = qkv_chunks(0)      # emits the x(0) DMA first
        emit_const_dmas_early()
        for c in chunks0:
            c()
        emit_const_dmas_late()
        if dbg:
            nc.sync.dma_start(dbg_q0[:, :], qropes[0][:, :])
            nc.sync.dma_start(dbg_k[:, 0:TCW], kpack[:, 0:TCW])

        # ---- main slots ----
        for s in range(TC):
            work = []
            if s + 1 < TC:
                work += qkv_chunks(s + 1)
            if s >= 1:
                work += oproj_chunks(s - 1, prev_pairs[0])
            qall = qropes[s]
            last_kj = 4 * s + 3
            n_iters = 4 * (4 * s + 4)
            W = len(work)
            ai = 0
            wi = 0

            def drain(quota):
                nonlocal wi
                while wi < W and wi < quota:
                    work[wi]()
                    wi += 1
            drain(2)   # prime: PE filler ahead of the first (possibly waiting) scores
            for p in range(4):
                augA = paug.tile([65, TCW], F32, tag="augA")
                augB = paug.tile([65, TCW], F32, tag="augB")
                for kj in range(4 * s + 4):
                    attn_iter(s, p, kj, qall, augA, augB, last_kj)
                    ai += 1
                    drain(2 + ai * W / n_iters)
                pack_end(s, p, augA, augB)
            drain(W)
            prev_pairs[0] = list(_pairs)

        # ---- epilogue: o_proj of the last chunk ----
        for c in oproj_chunks(TC - 1, prev_pairs[0]):
            c()
        if dbg:
            nc.sync.dma_start(dbg_v[:, :], vaug[:, :])

    nc.compile()
    return nc


def _host_prep(x, Wqkv, Wo):
    """Build per-core input maps. Returns list of 8 dicts."""
    # RoPE row permutation: within each 64-row head, reorder rows so the
    # half-split pair (r, r+32) lands at distance 16 inside one 32-block;
    # the on-device swap is then one full-width stream_shuffle.
    perm64 = np.r_[0:16, 32:48, 16:32, 48:64]

    invfreq = 1.0 / (ROPE_BASE ** (np.arange(0, HD, 2, dtype=np.float32) / HD))
    ang = np.arange(L, dtype=np.float32)[:, None] * invfreq[None, :]   # [L, 32]
    cos = np.cos(ang).T     # [32, L]
    sin = np.sin(ang).T
    cc = np.tile(cos, (4, 1)).astype(np.float32)                       # [128, L]
    sgn = np.repeat(np.array([-1.0, 1.0, -1.0, 1.0], np.float32), 32)
    ss = (np.tile(sin, (4, 1)) * sgn[:, None]).astype(np.float32)
    perm128 = np.concatenate([perm64, perm64 + 64])
    ccss = np.concatenate([cc[perm128], ss[perm128]], axis=1).astype(BF16)

    r = np.arange(128)[:, None]
    c = np.arange(TCW)[None, :]
    msk = np.concatenate(
        [(r + 128 * j <= c).astype(np.float32) for j in range(4)], axis=1
    ).astype(BF16)                                                     # [128, 2048]

    id2 = np.zeros((128, 64), np.float32)
    id2[:64] = np.eye(64, dtype=np.float32)
    id2[64:] = np.eye(64, dtype=np.float32)
    id2 = id2.astype(BF16)

    wq_part = Wqkv[:NH * HD].reshape(NH, HD, D)
    wk_part = Wqkv[NH * HD:NH * HD + NKV * HD].reshape(NKV, HD, D)
    wv_part = Wqkv[NH * HD + NKV * HD:].reshape(NKV, HD, D)

    in_maps = []
    for core in range(8):
        b, g = core // 4, core % 4
        xT = np.ascontiguousarray(x[b].T)                              # [D, L]
        xh = (xT.reshape(NKD, 128, TC, TCW).transpose(2, 1, 0, 3)
              .reshape(TC, 128, NKD * TCW)).astype(BF16)

        rows = []
        for p in range(4):
            rows.append(wq_part[8 * g + p][perm64])
            rows.append(wq_part[8 * g + 4 + p][perm64])
        rows.append(wk_part[2 * g][perm64]); rows.append(wk_part[2 * g + 1][perm64])
        rows.append(wv_part[2 * g]); rows.append(wv_part[2 * g + 1])
        Wc = np.concatenate(rows, axis=0)                              # [768, D]
        wq = (Wc.reshape(NOT, 128, NKD, 128).transpose(3, 2, 0, 1)
              .reshape(128, NKD * NOT * 128)).astype(BF16)

        cols = np.empty((4, 128), np.int64)
        for kt in range(4):
            cols[kt, :64] = (8 * g + kt) * HD + np.arange(64)
            cols[kt, 64:] = (8 * g + 4 + kt) * HD + np.arange(64)
        Woc = Wo.T[cols.reshape(-1)]                                   # [512, D]
        wo = (Woc.reshape(4, 128, NKD, 128).transpose(1, 0, 2, 3)
              .reshape(128, 4 * NKD * 128)).astype(BF16)

        in_maps.append(dict(xh=xh, wq=wq, wo=wo, ccss=ccss, msk=msk, id2=id2))
    return in_maps


def _get_nc():
    if "nc" not in _cached:
        _cached["nc"] = _build_nc()
    return _cached["nc"]


def run_sharded(x, Wqkv, Wo, trace=False):
    """Run on 8 cores; returns (out [B,L,D] float32, BassKernelResults)."""
    from concourse.bass_utils import run_bass_kernel_spmd
    nc = _get_nc()
    in_maps = _host_prep(np.asarray(x, np.float32), np.asarray(Wqkv, np.float32),
                         np.asarray(Wo, np.float32))
    res = run_bass_kernel_spmd(nc, in_maps, list(range(8)), trace=trace)
    out = np.zeros((B, L, D), np.float32)
    for core in range(8):
        b = core // 4
        P = res.results[core]["outp"].astype(np.float32).transpose(1, 2, 0, 3).reshape(D, L)
        out[b] += P.T
    return out, res


def kernel(x, Wqkv, Wo):
    out, _ = run_sharded(x, Wqkv, Wo, trace=False)
    return out


# revision 15
# speedup vs baseline: 1.0298x; 1.0044x over previous
"""TP(heads)xDP(batch) sharded causal GQA attention block for 8 trn2 cores.

Each core c handles batch b=c//4 and head group g=c%4 (8 q heads, 2 kv heads).
v2 pipeline (software-pipelined one chunk ahead, PE-stream interleaved):
  slot s emits: attention(s) iterations interleaved with qkv(s+1) matmul
  chunks and o_proj(s-1) tiles, so the PE never waits on RoPE/exp/norm.
  - RoPE rows are pre-permuted on the host so the half-swap is a single
    full-width (128-partition) stream_shuffle; all rope math in bf16.
  - scores_T[kj, qi] = k^T q  (bf16, 2 heads packed via PE row tiling)
  - exp on ScalarE (no max subtraction; scores ~N(0,1) after 1/8 scale),
    diagonal blocks N-restricted to the causal region.
  - out_aug[65, qi] = [v; ones]^T exp  (bf16; row 64 = softmax denominator)
  - normalize via DVE reciprocal_approx_fast + gpsimd partition_broadcast
  - partial_T[dout, t] = Wo_c^T attn (bf16 out); host sums the 8 partials.
"""
import sys
sys.path.insert(0, "/opt/trn_rl_repo")
from contextlib import ExitStack

import numpy as np
import ml_dtypes

B, L, D = 2, 2048, 2048
NH, NKV, HD = 32, 8, 64
ROPE_BASE = 10000.0
SCALE = HD ** -0.5
TC, TCW = 4, 512      # query/token chunks
NKD = 16              # d contraction tiles
NOT = 6               # output tiles per core (4 q packs, k pack, v pack)
NKJ = 16              # key tiles

BF16 = ml_dtypes.bfloat16

_cached = {}

# swap halves of 16 within each 32-partition quadrant (rope pair swap after
# the host-side row permutation)
SWAP16 = [(i + 16) % 32 for i in range(32)]


def _build_nc(dbg=False):
    import concourse.bacc as bacc
    import concourse.tile as tile
    import concourse.mybir as mybir
    from concourse import library_config

    F32 = mybir.dt.float32
    BF = mybir.dt.bfloat16
    AF = mybir.ActivationFunctionType

    nc = bacc.Bacc("TRN2", debug=False)
    xh_ap = nc.dram_tensor("xh", (TC, 128, NKD * TCW), BF, kind="ExternalInput").ap()
    wq_ap = nc.dram_tensor("wq", (128, NKD * NOT * 128), BF, kind="ExternalInput").ap()
    wo_ap = nc.dram_tensor("wo", (128, 4 * NKD * 128), BF, kind="ExternalInput").ap()
    ccss_ap = nc.dram_tensor("ccss", (128, 2 * L), BF, kind="ExternalInput").ap()
    msk_ap = nc.dram_tensor("msk", (128, 4 * TCW), BF, kind="ExternalInput").ap()
    id2_ap = nc.dram_tensor("id2", (128, 64), BF, kind="ExternalInput").ap()
    out_ap = nc.dram_tensor("outp", (TC, NKD, 128, TCW), BF, kind="ExternalOutput").ap()
    if dbg:
        dbg_q0 = nc.dram_tensor("dbg_q0", (128, 4 * TCW), BF, kind="ExternalOutput").ap()
        dbg_k = nc.dram_tensor("dbg_k", (128, L), BF, kind="ExternalOutput").ap()
        dbg_v = nc.dram_tensor("dbg_v", (128, 2 * NKJ * 65), BF, kind="ExternalOutput").ap()
        dbg_ep = nc.dram_tensor("dbg_ep", (128, 2 * TCW), BF, kind="ExternalOutput").ap()
        dbg_aug = nc.dram_tensor("dbg_aug", (65, TCW), F32, kind="ExternalOutput").ap()
        dbg_pair = nc.dram_tensor("dbg_pair", (128, TCW), BF, kind="ExternalOutput").ap()

    with tile.TileContext(nc) as tcx, ExitStack() as ctx:
        pc = ctx.enter_context(tcx.tile_pool(name="const", bufs=1))
        px = ctx.enter_context(tcx.tile_pool(name="x", bufs=2))
        pw = ctx.enter_context(tcx.tile_pool(name="work", bufs=1))
        psc = ctx.enter_context(tcx.tile_pool(name="psc", bufs=2, space="PSUM"))
        paug = ctx.enter_context(tcx.tile_pool(name="paug", bufs=1, space="PSUM"))
        pmm = ctx.enter_context(tcx.tile_pool(name="pmm", bufs=2, space="PSUM"))

        wq_t = pc.tile([128, NKD * NOT * 128], BF)
        wo_t = pc.tile([128, 4 * NKD * 128], BF)
        ccss_t = pc.tile([128, 2 * L], BF)
        msk_t = pc.tile([128, 4 * TCW], BF)
        id2_t = pc.tile([128, 64], BF)
        kpack = pc.tile([128, L], BF)            # k (2 kv heads stacked), RoPE'd
        vaug = pc.tile([128, 2 * NKJ * 65], BF)  # [v | ones] per (kv, kj)
        nc.vector.memset(vaug[:, 64::65], 1.0)   # ones columns

        with tcx.tile_critical():
            nc.gpsimd.load_library(library_config.attn)

        def emit_const_dmas_early():
            # contiguous wq pieces (dt-major): MM(dt) unblocks as piece dt//2 lands
            wqw = NKD * NOT * 128 // 8
            for i in range(8):
                nc.sync.dma_start(wq_t[:, i * wqw:(i + 1) * wqw],
                                  wq_ap[:, i * wqw:(i + 1) * wqw])
            # rope tables for chunk 0 only; rest comes later
            nc.sync.dma_start(ccss_t[:, 0:TCW], ccss_ap[:, 0:TCW])
            nc.sync.dma_start(ccss_t[:, L:L + TCW], ccss_ap[:, L:L + TCW])
            nc.sync.dma_start(id2_t[:, :], id2_ap[:, :])

        def emit_const_dmas_late():
            nc.sync.dma_start(msk_t[:, :], msk_ap[:, :])
            nc.sync.dma_start(ccss_t[:, TCW:L], ccss_ap[:, TCW:L])
            nc.sync.dma_start(ccss_t[:, L + TCW:2 * L], ccss_ap[:, L + TCW:2 * L])
            wow = 4 * NKD * 128 // 4
            for i in range(4):
                nc.sync.dma_start(wo_t[:, i * wow:(i + 1) * wow], wo_ap[:, i * wow:(i + 1) * wow])

        qropes = [None] * TC   # per-chunk roped q tiles
        _pairs = [None] * 4
        prev_pairs = [None]    # pairs of chunk s-1, consumed by o_proj in slot s

        def emit_x_dma(tci):
            xt = px.tile([128, NKD * TCW], BF, tag="xt")
            xw = NKD * TCW // 8
            with tcx.high_priority():
                for i in range(8):
                    nc.sync.dma_start(xt[:, i * xw:(i + 1) * xw],
                                      xh_ap[tci][:, i * xw:(i + 1) * xw])
            return xt

        def rope(dest_ap, raw, tci, nrep, w):
            """dest = raw*CC + swap16(raw)*SS over [128, w]; bf16 throughout."""
            with tcx.high_priority():
                swp = pw.tile([128, 4 * TCW], BF, tag="swp", bufs=2)
                nc.vector.stream_shuffle(swp[:, 0:w], raw[:, 0:w], SWAP16)
                cs = ccss_t[:, tci * TCW:(tci + 1) * TCW].unsqueeze(1).broadcast_to([128, nrep, TCW])
                sss = ccss_t[:, L + tci * TCW:L + (tci + 1) * TCW].unsqueeze(1).broadcast_to([128, nrep, TCW])
                r3 = raw[:, 0:w].rearrange("p (a b) -> p a b", a=nrep)
                s3 = swp[:, 0:w].rearrange("p (a b) -> p a b", a=nrep)
                nc.vector.tensor_mul(r3, r3, cs)
                nc.vector.tensor_mul(s3, s3, sss)
                nc.vector.tensor_add(dest_ap.rearrange("p (a b) -> p a b", a=nrep), r3, s3)

        def qkv_chunks(tci):
            """Return list of closures emitting the qkv projection for chunk
            tci: per-ot matmul groups (k, v first), then the q rope."""
            xt = emit_x_dma(tci)
            qraw = pw.tile([128, 4 * TCW], BF, tag="qraw", bufs=2)
            chunks = []

            def ot_chunk(ot):
                def emit():
                    ps = pmm.tile([128, TCW], F32, tag="mm")
                    for dt in range(NKD):
                        nc.tensor.matmul(
                            ps[:, :], wq_t[:, (dt * NOT + ot) * 128:(dt * NOT + ot + 1) * 128],
                            xt[:, dt * TCW:(dt + 1) * TCW],
                            start=(dt == 0), stop=(dt == NKD - 1))
                    if ot == 4:
                        kraw = pw.tile([128, TCW], BF, tag="kraw", bufs=2)
                        with tcx.high_priority():
                            nc.vector.tensor_copy(kraw[:, :], ps[:, :])
                        rope(kpack[:, tci * TCW:(tci + 1) * TCW], kraw, tci, 1, TCW)
                    elif ot == 5:
                        vch = pw.tile([128, TCW], BF, tag="vch", bufs=2)
                        with tcx.high_priority():
                            nc.vector.tensor_copy(vch[:, :], ps[:, :])
                        for j in range(2):
                            for jj in range(4):
                                kj = 4 * tci + jj
                                tp = pmm.tile([128, 64], BF, tag="mm")
                                nc.tensor.transpose(
                                    tp[:, :], vch[64 * j:64 * j + 64, jj * 128:(jj + 1) * 128],
                                    id2_t[64 * j:64 * j + 64, :])
                                col = (j * NKJ + kj) * 65
                                with tcx.high_priority():
                                    nc.vector.tensor_copy(vaug[:, col:col + 64], tp[:, :])
                    else:
                        with tcx.high_priority():
                            nc.vector.tensor_copy(qraw[:, ot * TCW:(ot + 1) * TCW], ps[:, :])
                return emit

            for ot in (4, 5, 0, 1, 2, 3):
                chunks.append(ot_chunk(ot))
            chunks.append(lambda: rope(qraw[:, :], qraw, tci, 4, 4 * TCW))
            qropes[tci] = qraw
            return chunks

        def oproj_chunks(otc, pairs_):
            """16 closures, one per dout tile (4 matmuls each). Evacuation
            alternates DVE/ScalarE so psum banks recycle twice as fast."""
            def dt_chunk(dt):
                def emit():
                    po = pmm.tile([128, TCW], F32, tag="mm")
                    for kt in range(4):
                        nc.tensor.matmul(
                            po[:, :], wo_t[:, (kt * NKD + dt) * 128:(kt * NKD + dt + 1) * 128],
                            pairs_[kt][:, :],
                            start=(kt == 0), stop=(kt == 3))
                    ev = pw.tile([128, TCW], BF, tag="ev", bufs=3)
                    if dt % 2:
                        nc.scalar.copy(ev[:, :], po[:, :])
                    else:
                        nc.vector.tensor_copy(ev[:, :], po[:, :])
                    nc.sync.dma_start(out_ap[otc, dt], ev[:, :])
                return emit
            return [dt_chunk(dt) for dt in range(NKD)]

        def attn_iter(s, p, kj, qall, augA, augB, last_kj):
            c0 = 128 * max(0, kj - 4 * s)          # causal column restriction
            n = TCW - c0
            qs0 = p * TCW + c0
            ks = slice(kj * 128, (kj + 1) * 128)
            scp = psc.tile([128, 2 * TCW], F32, tag="scp")
            nc.tensor.matmul(scp[:, c0:TCW], kpack[0:64, ks], qall[0:64, qs0:(p + 1) * TCW],
                             start=True, stop=True, tile_position=(0, 0))
            nc.tensor.matmul(scp[:, TCW + c0:2 * TCW], kpack[64:128, ks], qall[64:128, qs0:(p + 1) * TCW],
                             start=True, stop=True, tile_position=(64, 0))
            ep = pw.tile([128, 2 * TCW], BF, tag="ep", bufs=5)
            s3 = scp[:, :].rearrange("p (a b) -> p a b", a=2)[:, :, c0:TCW]
            e3 = ep[:, :].rearrange("p (a b) -> p a b", a=2)[:, :, c0:TCW]
            nc.scalar.activation(e3, s3, AF.Exp, scale=SCALE)
            dj = kj - 4 * s
            if dj >= 0:
                mb = (msk_t[:, dj * TCW + c0:(dj + 1) * TCW].unsqueeze(1)
                      .broadcast_to([128, 2, n]))
                with tcx.high_priority():
                    nc.vector.tensor_mul(e3, e3, mb)
            colA = (0 * NKJ + kj) * 65
            colB = (1 * NKJ + kj) * 65
            nc.tensor.matmul(augA[:, c0:TCW], vaug[:, colA:colA + 65], ep[:, c0:TCW],
                             start=(kj == 0), stop=(kj == last_kj))
            nc.tensor.matmul(augB[:, c0:TCW], vaug[:, colB:colB + 65], ep[:, TCW + c0:2 * TCW],
                             start=(kj == 0), stop=(kj == last_kj))
            if dbg and s == 0 and p == 0 and kj == 0:
                nc.sync.dma_start(dbg_ep[:, :], ep[:, :])

        def pack_end(s, p, augA, augB):
            """Evacuate aug psum, normalize (DVE recip), build the pair tile."""
            augSA = pw.tile([64, TCW], F32, tag="augSA", bufs=4)
            augSB = pw.tile([64, TCW], F32, tag="augSB", bufs=4)
            dnA = pw.tile([1, TCW], F32, tag="dn", bufs=9)
            dnB = pw.tile([1, TCW], F32, tag="dn", bufs=9)
            with tcx.high_priority():
                # dn rows first: they feed the long recip->broadcast pole
                nc.vector.tensor_copy(dnA[:, :], augA[64:65, :])
                nc.vector.tensor_copy(dnB[:, :], augB[64:65, :])
                nc.vector.tensor_copy(augSA[:, :], augA[0:64, :])
                nc.vector.tensor_copy(augSB[:, :], augB[0:64, :])
            rcA = pw.tile([1, TCW], F32, tag="rc", bufs=9)
            rcB = pw.tile([1, TCW], F32, tag="rc", bufs=9)
            with tcx.high_priority():
                nc.vector.reciprocal_approx_fast(rcA[:, :], dnA[:, :])
                nc.vector.reciprocal_approx_fast(rcB[:, :], dnB[:, :])
            pair = pw.tile([128, TCW], BF, tag="pair", bufs=8)
            bA = pw.tile([64, TCW], F32, tag="bA", bufs=2)
            nc.gpsimd.partition_broadcast(bA[:, :], rcA[:, :])
            nc.vector.tensor_mul(pair[0:64, :], augSA[0:64, :], bA[:, :])
            bB = pw.tile([64, TCW], F32, tag="bB", bufs=2)
            nc.gpsimd.partition_broadcast(bB[:, :], rcB[:, :])
            nc.vector.tensor_mul(pair[64:128, :], augSB[0:64, :], bB[:, :])
            if dbg and s == 0 and p == 0:
                nc.sync.dma_start(dbg_aug[0:64, :], augSA[:, :])
                nc.sync.dma_start(dbg_aug[64:65, :], dnA[:, :])
                nc.sync.dma_start(dbg_pair[:, :], pair[:, :])
            _pairs[p] = pair

        # ---- prologue: qkv + rope for chunk 0 ----
        chunks0 = qkv_chunks(0)      # emits the x(0) DMA first
        emit_const_dmas_early()
        for c in chunks0:
            c()
        emit_const_dmas_late()
        if dbg:
            nc.sync.dma_start(dbg_q0[:, :], qropes[0][:, :])
            nc.sync.dma_start(dbg_k[:, 0:TCW], kpack[:, 0:TCW])

        # ---- main slots ----
        for s in range(TC):
            work = []
            if s + 1 < TC:
                work += qkv_chunks(s + 1)
            if s >= 1:
                work += oproj_chunks(s - 1, prev_pairs[0])
            qall = qropes[s]
            last_kj = 4 * s + 3
            n_iters = 4 * (4 * s + 4)
            W = len(work)
            ai = 0
            wi = 0

            def drain(quota):
                nonlocal wi
                while wi < W and wi < quota:
                    work[wi]()
                    wi += 1
            drain(3)   # prime: PE filler ahead of the first (possibly waiting) scores
            for p in range(4):
                augA = paug.tile([65, TCW], F32, tag="augA")
                augB = paug.tile([65, TCW], F32, tag="augB")
                for kj in range(4 * s + 4):
                    attn_iter(s, p, kj, qall, augA, augB, last_kj)
                    ai += 1
                    drain(2 + ai * W / n_iters)
                pack_end(s, p, augA, augB)
            drain(W)
            prev_pairs[0] = list(_pairs)

        # ---- epilogue: o_proj of the last chunk ----
        for c in oproj_chunks(TC - 1, prev_pairs[0]):
            c()
        if dbg:
            nc.sync.dma_start(dbg_v[:, :], vaug[:, :])

    nc.compile()
    return nc


def _host_prep(x, Wqkv, Wo):
    """Build per-core input maps. Returns list of 8 dicts."""
    # RoPE row permutation: within each 64-row head, reorder rows so the
    # half-split pair (r, r+32) lands at distance 16 inside one 32-block;
    # the on-device swap is then one full-width stream_shuffle.
    perm64 = np.r_[0:16, 32:48, 16:32, 48:64]

    invfreq = 1.0 / (ROPE_BASE ** (np.arange(0, HD, 2, dtype=np.float32) / HD))
    ang = np.arange(L, dtype=np.float32)[:, None] * invfreq[None, :]   # [L, 32]
    cos = np.cos(ang).T     # [32, L]
    sin = np.sin(ang).T
    cc = np.tile(cos, (4, 1)).astype(np.float32)                       # [128, L]
    sgn = np.repeat(np.array([-1.0, 1.0, -1.0, 1.0], np.float32), 32)
    ss = (np.tile(sin, (4, 1)) * sgn[:, None]).astype(np.float32)
    perm128 = np.concatenate([perm64, perm64 + 64])
    ccss = np.concatenate([cc[perm128], ss[perm128]], axis=1).astype(BF16)

    r = np.arange(128)[:, None]
    c = np.arange(TCW)[None, :]
    msk = np.concatenate(
        [(r + 128 * j <= c).astype(np.float32) for j in range(4)], axis=1
    ).astype(BF16)                                                     # [128, 2048]

    id2 = np.zeros((128, 64), np.float32)
    id2[:64] = np.eye(64, dtype=np.float32)
    id2[64:] = np.eye(64, dtype=np.float32)
    id2 = id2.astype(BF16)

    wq_part = Wqkv[:NH * HD].reshape(NH, HD, D)
    wk_part = Wqkv[NH * HD:NH * HD + NKV * HD].reshape(NKV, HD, D)
    wv_part = Wqkv[NH * HD + NKV * HD:].reshape(NKV, HD, D)

    in_maps = []
    for core in range(8):
        b, g = core // 4, core % 4
        xT = np.ascontiguousarray(x[b].T)                              # [D, L]
        xh = (xT.reshape(NKD, 128, TC, TCW).transpose(2, 1, 0, 3)
              .reshape(TC, 128, NKD * TCW)).astype(BF16)

        rows = []
        for p in range(4):
            rows.append(wq_part[8 * g + p][perm64])
            rows.append(wq_part[8 * g + 4 + p][perm64])
        rows.append(wk_part[2 * g][perm64]); rows.append(wk_part[2 * g + 1][perm64])
        rows.append(wv_part[2 * g]); rows.append(wv_part[2 * g + 1])
        Wc = np.concatenate(rows, axis=0)                              # [768, D]
        wq = (Wc.reshape(NOT, 128, NKD, 128).transpose(3, 2, 0, 1)
              .reshape(128, NKD * NOT * 128)).astype(BF16)

        cols = np.empty((4, 128), np.int64)
        for kt in range(4):
            cols[kt, :64] = (8 * g + kt) * HD + np.arange(64)
            cols[kt, 64:] = (8 * g + 4 + kt) * HD + np.arange(64)
        Woc = Wo.T[cols.reshape(-1)]                                   # [512, D]
        wo = (Woc.reshape(4, 128, NKD, 128).transpose(1, 0, 2, 3)
              .reshape(128, 4 * NKD * 128)).astype(BF16)

        in_maps.append(dict(xh=xh, wq=wq, wo=wo, ccss=ccss, msk=msk, id2=id2))
    return in_maps


def _get_nc():
    if "nc" not in _cached:
        _cached["nc"] = _build_nc()
    return _cached["nc"]


def run_sharded(x, Wqkv, Wo, trace=False):
    """Run on 8 cores; returns (out [B,L,D] float32, BassKernelResults)."""
    from concourse.bass_utils import run_bass_kernel_spmd
    nc = _get_nc()
    in_maps = _host_prep(np.asarray(x, np.float32), np.asarray(Wqkv, np.float32),
                         np.asarray(Wo, np.float32))
    res = run_bass_kernel_spmd(nc, in_maps, list(range(8)), trace=trace)
    out = np.zeros((B, L, D), np.float32)
    for core in range(8):
        b = core // 4
        P = res.results[core]["outp"].astype(np.float32).transpose(1, 2, 0, 3).reshape(D, L)
        out[b] += P.T
    return out, res


def kernel(x, Wqkv, Wo):
    out, _ = run_sharded(x, Wqkv, Wo, trace=False)
    return out
